# revision 14
# baseline (speedup 1.0000x reference)
"""Trainium2 Bass kernel for single-head causal attention with projections.

Reference computation (B=4, T=4096, D=1024, H=64):
    qh = q @ Wq; kh = k @ Wk; vh = v @ Wv          # [B,T,H]
    S  = qh @ kh.T / sqrt(H)  (causal masked)       # [B,T,T]
    out = softmax(S) @ vh                           # [B,T,H]

Sharding: 8 cores = 4 batches x 2 query-halves. Each core owns half a
batch's queries (8 tiles of 256 rows, folded pairing so causal work is
balanced). K/V projection work is split between the two cores of a batch:
each core projects half of the kv positions and the pair exchanges the
small projected kh/vh tensors with an in-kernel AllGather (DRAM bounce),
instead of both cores re-reading and re-projecting the full K/V. A
position-padded schedule keeps all 8 cores on one identical SPMD program.

Performance structure:
  * every matmul operand is bf16 (1 PE cycle/row); PSUM accumulates f32.
  * the chip is power/ramp-throttled under full 8-core load, so chip-total
    PE rows and HBM traffic are minimized (kh/vh dedup) and the tensor
    engine is kept gapless so its p-state can ramp.
  * four phases; q-projection is interleaved with the k/v chains so the
    head is dense, and attention chunks are scheduled in the earliest
    phase where their q columns and exchanged kh/vh are ready. Each
    AllGather gets a full phase of slack before its consumers run.
  * causal tail masking uses a tiny per-core [128,4,512] pattern table
    (staircase/ones/zeros patterns) instead of a 2MB mask dump.
  * PV lags S^T by several kv chunks (exp output parked in SBUF) so the
    S -> exp -> mask -> PV cross-engine latency hides behind matmuls; the
    final all-attention phase borrows the idle projection PSUM banks for
    a 4-deep score pipeline.
  * output is stored transposed with the softmax denominator as row 65;
    the host does the final divide+transpose (2KB DMA lines, no PE work).
"""

import numpy as np

B, T, D, H = 4, 4096, 1024, 64
TILE = 256          # tq position tile
NPOS = 8            # q position tiles per core
DC = D // 128       # d chunks
NKV = T // 128      # kv chunks
NG = T // 1024      # kv stream groups (4); each core projects 512 of 1024
TQ = NPOS * TILE    # q rows per core
NPAIR = NPOS // 2
TH = T // 2         # kv columns projected per core

# per-position kv chunk counts (identical across cores): 32,28,...,4
COUNTS = [NKV - 4 * p for p in range(NPOS)]
# tile indices owned by a core: half 0 -> even tiles, half 1 -> odd tiles,
# position p maps to tile (14|15) - 2p so real extent <= COUNTS[p]
TILES_H0 = [14 - 2 * p for p in range(NPOS)]
TILES_H1 = [15 - 2 * p for p in range(NPOS)]
REPLICA_GROUPS = [[0, 1], [2, 3], [4, 5], [6, 7]]

_CACHE = {}


def _merge(base_events, attn_events):
    """Interleave two event lists proportionally."""
    nb, na = len(base_events), len(attn_events)
    if na == 0:
        return list(base_events)
    if nb == 0:
        return list(attn_events)
    out = []
    ai = 0
    acc = 0.0
    per = na / nb
    for ev in base_events:
        out.append(ev)
        acc += per
        while acc >= 1.0 and ai < na:
            out.append(attn_events[ai])
            ai += 1
            acc -= 1.0
    out.extend(attn_events[ai:])
    return out


def _zip2(a, b):
    """Strictly alternate two event lists, then leftovers."""
    out = []
    for x, y in zip(a, b):
        out.append(x)
        out.append(y)
    longer = a if len(a) > len(b) else b
    out.extend(longer[min(len(a), len(b)):])
    return out


def _build_program(counts, use_mask):
    import concourse.bacc as bacc
    import concourse.mybir as mybir
    import concourse.tile as tile
    from concourse.masks import make_identity

    f32 = mybir.dt.float32
    bf16 = mybir.dt.bfloat16

    nc = bacc.Bacc(None, target_bir_lowering=False, debug=False,
                   num_devices=8)
    qT = nc.declare_dram_parameter("qT", [D, TQ], bf16, isOutput=False)
    kT = nc.declare_dram_parameter("kT", [D, TH], bf16, isOutput=False)
    vT = nc.declare_dram_parameter("vT", [D, TH], bf16, isOutput=False)
    wq = nc.declare_dram_parameter("wq", [128, DC, H], bf16,
                                   isOutput=False)
    wk = nc.declare_dram_parameter("wk", [128, DC, H], bf16,
                                   isOutput=False)
    wv = nc.declare_dram_parameter("wv", [128, DC, H], bf16,
                                   isOutput=False)
    pat = nc.declare_dram_parameter("pat", [128, 4, 2 * TILE], bf16,
                                    isOutput=False)
    outT = nc.declare_dram_parameter("outT", [H + 1, TQ], f32,
                                     isOutput=True)

    qT_r = qT.rearrange("(c p) t -> c p t", p=128)
    kT_r = kT.rearrange("(c p) t -> c p t", p=128)
    vT_r = vT.rearrange("(c p) t -> c p t", p=128)
    scale = 1.0 / float(np.sqrt(H))

    with tile.TileContext(nc) as tc:
        with (
            tc.tile_pool(name="singles", bufs=1) as singles,
            tc.tile_pool(name="stream", bufs=16) as stream,
            tc.tile_pool(name="psb", bufs=8) as psbp,
            tc.tile_pool(name="dram", bufs=2, space="DRAM") as dram,
            tc.tile_pool(name="proj_ps", bufs=2, space="PSUM") as pps,
            tc.tile_pool(name="st_ps", bufs=2, space="PSUM") as stps,
            tc.tile_pool(name="pv_ps", bufs=1, space="PSUM") as pvp,
        ):
            wq_sb = singles.tile([128, DC, H], bf16, tag="wq")
            wk_sb = singles.tile([128, DC, H], bf16, tag="wk")
            wv_sb = singles.tile([128, DC, H], bf16, tag="wv")
            nc.sync.dma_start(out=wq_sb, in_=wq[:, :, :])
            nc.sync.dma_start(out=wk_sb, in_=wk[:, :, :])
            nc.sync.dma_start(out=wv_sb, in_=wv[:, :, :])

            qhT = singles.tile([64, TQ], bf16, tag="qhT")
            khT = singles.tile([64, T], bf16, tag="khT")
            vh1 = singles.tile([128, NKV, H + 1], bf16, tag="vh1")

            ident32 = singles.tile([128, 128], f32, tag="id32")
            make_identity(nc, ident32)
            identb = singles.tile([128, 128], bf16, tag="idb")
            nc.vector.tensor_copy(identb, ident32)
            pat_sb = singles.tile([128, 4, 2 * TILE], bf16, tag="pat")
            nc.sync.dma_start(out=pat_sb, in_=pat[:, :, :])
            nc.vector.memset(vh1[:, :, H:H + 1], 1.0)

            # per-pair PV^T accumulators: [65, 512] = one PSUM bank each
            pvt = [pvp.tile([65, 2 * TILE], f32, tag=f"pv{j}",
                            name=f"pvt{j}")
                   for j in range(NPAIR)]

            # ---- projection chains -----------------------------------
            def stream_dma(t, src, split):
                if split:
                    for s in range(4):
                        nc.sync.dma_start(out=t[:, s * 128:(s + 1) * 128],
                                          in_=src[:, s * 128:(s + 1) * 128])
                else:
                    nc.sync.dma_start(out=t, in_=src)

            def qchain(qg, half, split_first=False):
                ph = pps.tile([64, 512], f32, tag="ph", name=f"phq{qg}{half}")
                col0 = qg * 1024 + half * 512
                evs = []

                def step(c):
                    def go():
                        t = stream.tile([128, 512], bf16, tag="qkv")
                        stream_dma(t, qT_r[c, :, col0:col0 + 512],
                                   split_first and c == 0)
                        nc.tensor.matmul(ph, wq_sb[:, c, :], t,
                                         start=(c == 0), stop=(c == DC - 1))
                    return go
                evs.extend(step(c) for c in range(DC))

                def evict():
                    nc.vector.tensor_copy(qhT[:, col0:col0 + 512], ph)
                evs.append(evict)
                return evs

            def kchain(gg, split_first=False):
                ph = pps.tile([64, 512], f32, tag="ph", name=f"phk{gg}")
                evs = []

                def step(c):
                    def go():
                        t = stream.tile([128, 512], bf16, tag="qkv")
                        stream_dma(t, kT_r[c, :, gg * 512:(gg + 1) * 512],
                                   split_first and c == 0)
                        nc.tensor.matmul(ph, wk_sb[:, c, :], t,
                                         start=(c == 0), stop=(c == DC - 1))
                    return go
                evs.extend(step(c) for c in range(DC))

                def exchange():
                    khl = stream.tile([64, 512], bf16, tag="khl")
                    nc.vector.tensor_copy(khl, ph)
                    kbi = dram.tile([64, 512], bf16, tag="kbi", name="kbi")
                    nc.sync.dma_start(out=kbi[:], in_=khl)
                    kbo = dram.tile([128, 512], bf16, tag="kbo", name="kbo")
                    nc.gpsimd.collective_compute(
                        "AllGather", mybir.AluOpType.bypass,
                        replica_groups=REPLICA_GROUPS,
                        ins=[kbi.opt()], outs=[kbo.opt()])
                    g0 = gg * 1024
                    nc.sync.dma_start(out=khT[:, g0:g0 + 512],
                                      in_=kbo[0:64, :])
                    nc.sync.dma_start(out=khT[:, g0 + 512:g0 + 1024],
                                      in_=kbo[64:128, :])
                evs.append(exchange)
                return evs

            def vchain(gg):
                ph = pps.tile([64, 512], f32, tag="ph", name=f"phv{gg}")
                evs = []

                def step(c):
                    def go():
                        t = stream.tile([128, 512], bf16, tag="qkv")
                        nc.sync.dma_start(
                            out=t, in_=vT_r[c, :, gg * 512:(gg + 1) * 512])
                        nc.tensor.matmul(ph, wv_sb[:, c, :], t,
                                         start=(c == 0), stop=(c == DC - 1))
                    return go
                evs.extend(step(c) for c in range(DC))

                def fold():
                    vtmp = stream.tile([64, 512], bf16, tag="vtmp")
                    nc.vector.tensor_copy(vtmp, ph)
                    vst = stream.tile([128, 4, H], bf16, tag="vst")
                    for s in range(4):
                        ptr = stps.tile([128, H], bf16, tag="st")
                        nc.tensor.transpose(
                            ptr, vtmp[:, s * 128:(s + 1) * 128],
                            identb[:64, :64])
                        nc.vector.tensor_copy(vst[:, s, :], ptr)
                    vbi = dram.tile([128, 4 * H], bf16, tag="vbi", name="vbi")
                    nc.sync.dma_start(out=vbi[:], in_=vst)
                    vbo = dram.tile([256, 4 * H], bf16, tag="vbo", name="vbo")
                    nc.gpsimd.collective_compute(
                        "AllGather", mybir.AluOpType.bypass,
                        replica_groups=REPLICA_GROUPS,
                        ins=[vbi.opt()], outs=[vbo.opt()])
                    vbo_r = vbo.rearrange("(g p) (s h) -> g p s h", p=128, s=4)
                    for half in range(2):
                        nc.sync.dma_start(
                            out=vh1[:, 8 * gg + 4 * half:
                                    8 * gg + 4 * half + 4, 0:H],
                            in_=vbo_r[half, :, :, :])
                evs.append(fold)
                return evs

            # ---- attention ------------------------------------------
            def attn_s(m, j, box, pool):
                wide = counts[2 * j + 1] > m
                width = 2 * TILE if wide else TILE
                tag = "st" if pool is stps else "ph"

                def go():
                    stp = pool.tile([128, 2 * TILE], f32, tag=tag,
                                    name=f"s{m}_{j}")
                    nc.tensor.matmul(
                        stp[:, :width], khT[:, m * 128:(m + 1) * 128],
                        qhT[:, 2 * j * TILE:2 * j * TILE + width],
                        start=True, stop=True)
                    psb = psbp.tile([128, 2 * TILE], bf16, tag="p")
                    nc.scalar.activation(
                        psb[:, :width], stp[:, :width],
                        mybir.ActivationFunctionType.Exp, scale=scale)
                    if use_mask:
                        cL, cR = counts[2 * j], counts[2 * j + 1]
                        if m >= cL - 4:
                            nc.vector.tensor_mul(
                                psb[:, :TILE], psb[:, :TILE],
                                pat_sb[:, m - cL + 4, :TILE])
                        if wide and m >= cR - 4:
                            nc.vector.tensor_mul(
                                psb[:, TILE:2 * TILE],
                                psb[:, TILE:2 * TILE],
                                pat_sb[:, m - cR + 4, :TILE])
                    box.append((psb, width))
                return go

            def attn_pv(m, j, box):
                def go():
                    psb, width = box[0]
                    nc.tensor.matmul(
                        pvt[j][:, :width], vh1[:, m, :], psb[:, :width],
                        start=(m == 0), stop=(m == counts[2 * j] - 1),
                        skip_group_check=True)
                return go

            def attn_events(chunks, lag=6, dual=False):
                evs = []
                pend = []
                for idx, (m, j) in enumerate(chunks):
                    pool = pps if (dual and idx % 2) else stps
                    box = []
                    evs.append(attn_s(m, j, box, pool))
                    pend.append(attn_pv(m, j, box))
                    if len(pend) > lag:
                        evs.append(pend.pop(0))
                evs.extend(pend)
                return evs

            def drain(j):
                def go():
                    dsb = stream.tile([65, 2 * TILE], f32, tag="dsb")
                    nc.vector.tensor_copy(dsb, pvt[j])
                    for ps, pe in ((0, 33), (33, 65)):
                        for cs in (0, 256):
                            nc.sync.dma_start(
                                out=outT[ps:pe, j * 512 + cs:
                                         j * 512 + cs + 256],
                                in_=dsb[ps:pe, cs:cs + 256])
                return [go]

            # chunk lists per phase (j-major in P1 so early pairs never
            # wait on later q columns; m-major later so late kv groups
            # are consumed late, after their exchange completes)
            p1_chunks = [(m, j) for j in (0, 1, 2)
                         for m in range(0, min(8, counts[2 * j]))]
            p2_chunks = ([(m, 3) for m in range(0, min(8, counts[6]))]
                         + [(m, j) for m in range(8, 16)
                            for j in range(NPAIR) if counts[2 * j] > m])
            p3_chunks = [(m, j) for m in range(16, NKV)
                         for j in range(NPAIR) if counts[2 * j] > m]

            # ---- phase 0: q halves 0E/0O with kv group 0 ----
            p0 = (_zip2(kchain(0, split_first=True),
                        qchain(0, 0, split_first=True))
                  + _zip2(vchain(0), qchain(0, 1)))
            for ev in p0:
                ev()

            # ---- phase 1: q halves 1E/1O + kv group 1 + attn P1 ----
            p1 = (_zip2(kchain(1), qchain(1, 0))
                  + _zip2(vchain(1), qchain(1, 1)))
            for ev in _merge(p1, attn_events(p1_chunks)):
                ev()

            # ---- phase 2: kv groups 2 and 3 + attn P2 ----
            p2 = kchain(2) + vchain(2) + vchain(3) + kchain(3)
            for ev in _merge(p2, attn_events(p2_chunks)):
                ev()

            # ---- phase 3: remaining attention + drains ----
            for ev in _merge(drain(3) + drain(2),
                             attn_events(p3_chunks, dual=True)):
                ev()
            for ev in drain(1) + drain(0):
                ev()
    nc.compile()
    return nc


def _get_program(key, counts, use_mask):
    if key not in _CACHE:
        _CACHE[key] = _build_program(counts, use_mask)
    return _CACHE[key]


def _numpy_fallback(q, k, v, mask, Wq, Wk, Wv):
    qh = q.astype(np.float32) @ Wq
    kh = k.astype(np.float32) @ Wk
    vh = v.astype(np.float32) @ Wv
    out = np.empty((B, T, H), np.float32)
    neg = np.float32(-1e30)
    for b in range(B):
        s = (qh[b] @ kh[b].T) / np.float32(np.sqrt(H))
        s = np.where(mask == 0, neg, s)
        s = s - s.max(axis=-1, keepdims=True)
        e = np.exp(s)
        w = e / e.sum(axis=-1, keepdims=True)
        out[b] = w @ vh[b]
    return out


def _w_layout(w, np_in):
    """[D, H] -> [128, DC, H]: partition-major layout for dense DMA."""
    return np.ascontiguousarray(
        w.reshape(DC, 128, H).transpose(1, 0, 2), np_in)


def _make_pat(half):
    """[128, 4, 512] tail-mask patterns; only the first 256 cols are used."""
    tk = np.arange(128)[:, None]
    c = np.arange(2 * TILE)[None, :]
    stair0 = (c >= tk).astype(np.float32)
    stair1 = (c >= 128 + tk).astype(np.float32)
    ones = np.ones((128, 2 * TILE), np.float32)
    zeros = np.zeros((128, 2 * TILE), np.float32)
    if half == 0:
        pats = [stair0, stair1, zeros, zeros]
    else:
        pats = [ones, ones, stair0, stair1]
    return np.stack(pats, axis=1)  # [128, 4, 512]


def _make_in_maps(q, k, v, mask, Wq, Wk, Wv, counts, apply_tail, np_in):
    # kv half-columns owned by core half h: [gg*1024 + h*512, +512) per gg
    half_idx = [
        np.concatenate([np.arange(gg * 1024 + h * 512,
                                  gg * 1024 + h * 512 + 512)
                        for gg in range(NG)])
        for h in range(2)
    ]
    in_maps = []
    metas = []
    for core in range(8):
        b, h = divmod(core, 2)
        tiles = TILES_H0 if h == 0 else TILES_H1
        qT_slab = np.concatenate(
            [q[b, i * TILE:(i + 1) * TILE, :].T for i in tiles], axis=1)
        pat = _make_pat(h) if apply_tail else np.ones(
            (128, 4, 2 * TILE), np.float32)
        kTb = k[b].T
        vTb = v[b].T
        im = {
            "qT": np.ascontiguousarray(qT_slab, np_in),
            "kT": np.ascontiguousarray(kTb[:, half_idx[h]], np_in),
            "vT": np.ascontiguousarray(vTb[:, half_idx[h]], np_in),
            "wq": _w_layout(Wq, np_in), "wk": _w_layout(Wk, np_in),
            "wv": _w_layout(Wv, np_in),
            "pat": np.ascontiguousarray(pat, np_in),
        }
        in_maps.append(im)
        metas.append((b, tiles))
    return in_maps, metas


def kernel(q, k, v, mask, Wq, Wk, Wv):
    from concourse.bass_utils import run_bass_kernel_spmd
    import ml_dtypes

    q = np.ascontiguousarray(q, np.float32)
    k = np.ascontiguousarray(k, np.float32)
    v = np.ascontiguousarray(v, np.float32)
    Wq = np.ascontiguousarray(Wq, np.float32)
    Wk = np.ascontiguousarray(Wk, np.float32)
    Wv = np.ascontiguousarray(Wv, np.float32)
    mask = np.asarray(mask)

    is_tril = bool((mask == np.tril(np.ones((T, T), mask.dtype))).all())
    is_ones = bool((mask == 1).all())
    if not (is_tril or is_ones):
        return _numpy_fallback(q, k, v, mask, Wq, Wk, Wv)

    np_in = ml_dtypes.bfloat16
    counts = COUNTS if is_tril else [NKV] * NPOS
    nc = _get_program(("v6", is_tril), counts, is_tril)

    in_maps, metas = _make_in_maps(
        q, k, v, mask, Wq, Wk, Wv, counts, is_tril, np_in)
    res = run_bass_kernel_spmd(nc, in_maps, list(range(8)))

    out = np.empty((B, T, H), np.float32)
    for c in range(8):
        b, tiles = metas[c]
        oc = res.results[c]["outT"]  # [H+1, TQ]: rows 0..H-1 num, row H den
        slab = (oc[:H, :] / oc[H:H + 1, :]).T  # [TQ, H]
        for p, i in enumerate(tiles):
            out[b, i * TILE:(i + 1) * TILE, :] = \
                slab[p * TILE:(p + 1) * TILE, :]
    return out


# revision 16
# speedup vs baseline: 1.0096x; 1.0096x over previous
"""Trainium2 Bass kernel for single-head causal attention with projections.

Reference computation (B=4, T=4096, D=1024, H=64):
    qh = q @ Wq; kh = k @ Wk; vh = v @ Wv          # [B,T,H]
    S  = qh @ kh.T / sqrt(H)  (causal masked)       # [B,T,T]
    out = softmax(S) @ vh                           # [B,T,H]

Sharding: 8 cores = 4 batches x 2 query-halves. Each core owns half a
batch's queries (8 tiles of 256 rows, folded pairing so causal work is
balanced). K/V projection work is split between the two cores of a batch:
each core projects half of the kv positions and the pair exchanges the
small projected kh/vh tensors with an in-kernel AllGather (DRAM bounce),
instead of both cores re-reading and re-projecting the full K/V. A
position-padded schedule keeps all 8 cores on one identical SPMD program.

Performance structure:
  * every matmul operand is bf16 (1 PE cycle/row); PSUM accumulates f32.
  * the chip is power/ramp-throttled under full 8-core load, so chip-total
    PE rows and HBM traffic are minimized (kh/vh dedup) and the tensor
    engine is kept gapless so its p-state can ramp.
  * four phases; q-projection is interleaved with the k/v chains so the
    head is dense, and attention chunks are scheduled in the earliest
    phase where their q columns and exchanged kh/vh are ready. Each
    AllGather gets a full phase of slack before its consumers run.
  * causal tail masking uses a tiny per-core [128,4,512] pattern table
    (staircase/ones/zeros patterns) instead of a 2MB mask dump.
  * PV lags S^T by several kv chunks (exp output parked in SBUF) so the
    S -> exp -> mask -> PV cross-engine latency hides behind matmuls; the
    final all-attention phase borrows the idle projection PSUM banks for
    a 4-deep score pipeline.
  * output is stored transposed with the softmax denominator as row 65;
    the host does the final divide+transpose (2KB DMA lines, no PE work).
"""

import numpy as np

B, T, D, H = 4, 4096, 1024, 64
TILE = 256          # tq position tile
NPOS = 8            # q position tiles per core
DC = D // 128       # d chunks
NKV = T // 128      # kv chunks
NG = T // 1024      # kv stream groups (4); each core projects 512 of 1024
TQ = NPOS * TILE    # q rows per core
NPAIR = NPOS // 2
TH = T // 2         # kv columns projected per core

# per-position kv chunk counts (identical across cores): 32,28,...,4
COUNTS = [NKV - 4 * p for p in range(NPOS)]
# tile indices owned by a core: half 0 -> even tiles, half 1 -> odd tiles,
# position p maps to tile (14|15) - 2p so real extent <= COUNTS[p]
TILES_H0 = [14 - 2 * p for p in range(NPOS)]
TILES_H1 = [15 - 2 * p for p in range(NPOS)]
REPLICA_GROUPS = [[0, 1], [2, 3], [4, 5], [6, 7]]

_CACHE = {}


def _merge(base_events, attn_events):
    """Interleave two event lists proportionally."""
    nb, na = len(base_events), len(attn_events)
    if na == 0:
        return list(base_events)
    if nb == 0:
        return list(attn_events)
    out = []
    ai = 0
    acc = 0.0
    per = na / nb
    for ev in base_events:
        out.append(ev)
        acc += per
        while acc >= 1.0 and ai < na:
            out.append(attn_events[ai])
            ai += 1
            acc -= 1.0
    out.extend(attn_events[ai:])
    return out


def _zip2(a, b):
    """Strictly alternate two event lists, then leftovers."""
    out = []
    for x, y in zip(a, b):
        out.append(x)
        out.append(y)
    longer = a if len(a) > len(b) else b
    out.extend(longer[min(len(a), len(b)):])
    return out


def _build_program(counts, use_mask):
    import concourse.bacc as bacc
    import concourse.mybir as mybir
    import concourse.tile as tile
    from concourse.masks import make_identity

    f32 = mybir.dt.float32
    bf16 = mybir.dt.bfloat16

    nc = bacc.Bacc(None, target_bir_lowering=False, debug=False,
                   num_devices=8)
    qT = nc.declare_dram_parameter("qT", [D, TQ], bf16, isOutput=False)
    kT = nc.declare_dram_parameter("kT", [D, TH], bf16, isOutput=False)
    vT = nc.declare_dram_parameter("vT", [D, TH], bf16, isOutput=False)
    wq = nc.declare_dram_parameter("wq", [128, DC, H], bf16,
                                   isOutput=False)
    wk = nc.declare_dram_parameter("wk", [128, DC, H], bf16,
                                   isOutput=False)
    wv = nc.declare_dram_parameter("wv", [128, DC, H], bf16,
                                   isOutput=False)
    pat = nc.declare_dram_parameter("pat", [128, 4, 2 * TILE], bf16,
                                    isOutput=False)
    outT = nc.declare_dram_parameter("outT", [H + 1, TQ], f32,
                                     isOutput=True)

    qT_r = qT.rearrange("(c p) t -> c p t", p=128)
    kT_r = kT.rearrange("(c p) t -> c p t", p=128)
    vT_r = vT.rearrange("(c p) t -> c p t", p=128)
    scale = 1.0 / float(np.sqrt(H))

    with tile.TileContext(nc) as tc:
        with (
            tc.tile_pool(name="singles", bufs=1) as singles,
            tc.tile_pool(name="stream", bufs=16) as stream,
            tc.tile_pool(name="psb", bufs=8) as psbp,
            tc.tile_pool(name="dram", bufs=2, space="DRAM") as dram,
            tc.tile_pool(name="proj_ps", bufs=2, space="PSUM") as pps,
            tc.tile_pool(name="st_ps", bufs=2, space="PSUM") as stps,
            tc.tile_pool(name="pv_ps", bufs=1, space="PSUM") as pvp,
        ):
            wq_sb = singles.tile([128, DC, H], bf16, tag="wq")
            wk_sb = singles.tile([128, DC, H], bf16, tag="wk")
            wv_sb = singles.tile([128, DC, H], bf16, tag="wv")
            nc.sync.dma_start(out=wq_sb, in_=wq[:, :, :])
            nc.sync.dma_start(out=wk_sb, in_=wk[:, :, :])
            nc.sync.dma_start(out=wv_sb, in_=wv[:, :, :])

            qhT = singles.tile([64, TQ], bf16, tag="qhT")
            khT = singles.tile([64, T], bf16, tag="khT")
            vh1 = singles.tile([128, NKV, H + 1], bf16, tag="vh1")

            ident32 = singles.tile([128, 128], f32, tag="id32")
            make_identity(nc, ident32)
            identb = singles.tile([128, 128], bf16, tag="idb")
            nc.vector.tensor_copy(identb, ident32)
            pat_sb = singles.tile([128, 4, 2 * TILE], bf16, tag="pat")
            nc.sync.dma_start(out=pat_sb, in_=pat[:, :, :])
            nc.vector.memset(vh1[:, :, H:H + 1], 1.0)

            # per-pair PV^T accumulators: [65, 512] = one PSUM bank each
            pvt = [pvp.tile([65, 2 * TILE], f32, tag=f"pv{j}",
                            name=f"pvt{j}")
                   for j in range(NPAIR)]

            # ---- projection chains -----------------------------------
            def stream_dma(t, src, split, eng=None):
                eng = eng or nc.sync
                if split:
                    for s in range(4):
                        eng.dma_start(out=t[:, s * 128:(s + 1) * 128],
                                      in_=src[:, s * 128:(s + 1) * 128])
                else:
                    eng.dma_start(out=t, in_=src)

            def qchain(qg, half, split_first=False):
                ph = pps.tile([64, 512], f32, tag="ph", name=f"phq{qg}{half}")
                col0 = qg * 1024 + half * 512
                evs = []

                def step(c):
                    def go():
                        t = stream.tile([128, 512], bf16, tag="qkv")
                        stream_dma(t, qT_r[c, :, col0:col0 + 512],
                                   split_first and c == 0, eng=nc.scalar)
                        nc.tensor.matmul(ph, wq_sb[:, c, :], t,
                                         start=(c == 0), stop=(c == DC - 1))
                    return go
                evs.extend(step(c) for c in range(DC))

                def evict():
                    nc.vector.tensor_copy(qhT[:, col0:col0 + 512], ph)
                evs.append(evict)
                return evs

            def kchain(gg, split_first=False):
                ph = pps.tile([64, 512], f32, tag="ph", name=f"phk{gg}")
                evs = []

                def step(c):
                    def go():
                        t = stream.tile([128, 512], bf16, tag="qkv")
                        stream_dma(t, kT_r[c, :, gg * 512:(gg + 1) * 512],
                                   split_first and c == 0)
                        nc.tensor.matmul(ph, wk_sb[:, c, :], t,
                                         start=(c == 0), stop=(c == DC - 1))
                    return go
                evs.extend(step(c) for c in range(DC))

                def exchange():
                    khl = stream.tile([64, 512], bf16, tag="khl")
                    nc.vector.tensor_copy(khl, ph)
                    kbi = dram.tile([64, 512], bf16, tag="kbi", name="kbi")
                    nc.sync.dma_start(out=kbi[:], in_=khl)
                    kbo = dram.tile([128, 512], bf16, tag="kbo", name="kbo")
                    nc.gpsimd.collective_compute(
                        "AllGather", mybir.AluOpType.bypass,
                        replica_groups=REPLICA_GROUPS,
                        ins=[kbi.opt()], outs=[kbo.opt()])
                    g0 = gg * 1024
                    nc.sync.dma_start(out=khT[:, g0:g0 + 512],
                                      in_=kbo[0:64, :])
                    nc.sync.dma_start(out=khT[:, g0 + 512:g0 + 1024],
                                      in_=kbo[64:128, :])
                evs.append(exchange)
                return evs

            def vchain(gg):
                ph = pps.tile([64, 512], f32, tag="ph", name=f"phv{gg}")
                evs = []

                def step(c):
                    def go():
                        t = stream.tile([128, 512], bf16, tag="qkv")
                        nc.sync.dma_start(
                            out=t, in_=vT_r[c, :, gg * 512:(gg + 1) * 512])
                        nc.tensor.matmul(ph, wv_sb[:, c, :], t,
                                         start=(c == 0), stop=(c == DC - 1))
                    return go
                evs.extend(step(c) for c in range(DC))

                def fold():
                    vtmp = stream.tile([64, 512], bf16, tag="vtmp")
                    nc.vector.tensor_copy(vtmp, ph)
                    vst = stream.tile([128, 4, H], bf16, tag="vst")
                    for s in range(4):
                        ptr = stps.tile([128, H], bf16, tag="st")
                        nc.tensor.transpose(
                            ptr, vtmp[:, s * 128:(s + 1) * 128],
                            identb[:64, :64])
                        nc.vector.tensor_copy(vst[:, s, :], ptr)
                    vbi = dram.tile([128, 4 * H], bf16, tag="vbi", name="vbi")
                    nc.sync.dma_start(out=vbi[:], in_=vst)
                    vbo = dram.tile([256, 4 * H], bf16, tag="vbo", name="vbo")
                    nc.gpsimd.collective_compute(
                        "AllGather", mybir.AluOpType.bypass,
                        replica_groups=REPLICA_GROUPS,
                        ins=[vbi.opt()], outs=[vbo.opt()])
                    vbo_r = vbo.rearrange("(g p) (s h) -> g p s h", p=128, s=4)
                    for half in range(2):
                        nc.sync.dma_start(
                            out=vh1[:, 8 * gg + 4 * half:
                                    8 * gg + 4 * half + 4, 0:H],
                            in_=vbo_r[half, :, :, :])
                evs.append(fold)
                return evs

            # ---- attention ------------------------------------------
            def attn_s(m, j, box, pool):
                wide = counts[2 * j + 1] > m
                width = 2 * TILE if wide else TILE
                tag = "st" if pool is stps else "ph"

                def go():
                    stp = pool.tile([128, 2 * TILE], f32, tag=tag,
                                    name=f"s{m}_{j}")
                    nc.tensor.matmul(
                        stp[:, :width], khT[:, m * 128:(m + 1) * 128],
                        qhT[:, 2 * j * TILE:2 * j * TILE + width],
                        start=True, stop=True)
                    psb = psbp.tile([128, 2 * TILE], bf16, tag="p")
                    nc.scalar.activation(
                        psb[:, :width], stp[:, :width],
                        mybir.ActivationFunctionType.Exp, scale=scale)
                    if use_mask:
                        cL, cR = counts[2 * j], counts[2 * j + 1]
                        if m >= cL - 4:
                            nc.vector.tensor_mul(
                                psb[:, :TILE], psb[:, :TILE],
                                pat_sb[:, m - cL + 4, :TILE])
                        if wide and m >= cR - 4:
                            nc.vector.tensor_mul(
                                psb[:, TILE:2 * TILE],
                                psb[:, TILE:2 * TILE],
                                pat_sb[:, m - cR + 4, :TILE])
                    box.append((psb, width))
                return go

            def attn_pv(m, j, box, stop):
                def go():
                    psb, width = box[0]
                    nc.tensor.matmul(
                        pvt[j][:, :width], vh1[:, m, :], psb[:, :width],
                        start=(m == 0), stop=stop,
                        skip_group_check=True)
                return go

            def attn_events(chunks, lag=6, dual=False):
                evs = []
                pend = []
                for idx, (m, j) in enumerate(chunks):
                    pool = pps if (dual and idx % 2) else stps
                    box = []
                    evs.append(attn_s(m, j, box, pool))
                    pend.append(attn_pv(m, j, box, last_chunk[j] == (m, j)))
                    if len(pend) > lag:
                        evs.append(pend.pop(0))
                evs.extend(pend)
                return evs

            def drain(j):
                def go():
                    dsb = stream.tile([65, 2 * TILE], f32, tag="dsb")
                    nc.vector.tensor_copy(dsb, pvt[j])
                    for ps, pe in ((0, 33), (33, 65)):
                        for cs in (0, 256):
                            nc.sync.dma_start(
                                out=outT[ps:pe, j * 512 + cs:
                                         j * 512 + cs + 256],
                                in_=dsb[ps:pe, cs:cs + 256])
                return [go]

            # chunk lists per phase (j-major in P1 so early pairs never
            # wait on later q columns; m-major later so late kv groups
            # are consumed late, after their exchange completes)
            p1_chunks = [(m, j) for j in (0, 1, 2)
                         for m in range(0, min(8, counts[2 * j]))]
            p2_chunks = ([(m, 3) for m in range(0, min(8, counts[6]))]
                         + [(m, j) for m in range(8, 16)
                            for j in range(NPAIR) if counts[2 * j] > m])
            p3_chunks = ([(m, j) for m in range(24, NKV)
                          for j in range(NPAIR) if counts[2 * j] > m]
                         + [(m, j) for m in range(16, 24)
                            for j in range(NPAIR) if counts[2 * j] > m])
            last_chunk = {}
            for ch in p1_chunks + p2_chunks + p3_chunks:
                last_chunk[ch[1]] = ch

            # ---- phase 0: q halves 0E/0O with kv group 0 ----
            p0 = (_zip2(kchain(0, split_first=True),
                        qchain(0, 0, split_first=True))
                  + _zip2(vchain(0), qchain(0, 1)))
            for ev in p0:
                ev()

            # ---- phase 1: q halves 1E/1O + kv group 1 + attn P1 ----
            p1 = (_zip2(kchain(1), qchain(1, 0))
                  + _zip2(vchain(1), qchain(1, 1)))
            for ev in _merge(p1, attn_events(p1_chunks)):
                ev()

            # ---- phase 2: kv groups 2 and 3 + attn P2 ----
            p2 = kchain(3) + vchain(3) + kchain(2) + vchain(2)
            for ev in _merge(p2, attn_events(p2_chunks)):
                ev()

            # ---- phase 3: remaining attention + drains ----
            for ev in _merge(drain(3) + drain(2),
                             attn_events(p3_chunks, dual=True)):
                ev()
            for ev in drain(1) + drain(0):
                ev()
    nc.compile()
    return nc


def _get_program(key, counts, use_mask):
    if key not in _CACHE:
        _CACHE[key] = _build_program(counts, use_mask)
    return _CACHE[key]


def _numpy_fallback(q, k, v, mask, Wq, Wk, Wv):
    qh = q.astype(np.float32) @ Wq
    kh = k.astype(np.float32) @ Wk
    vh = v.astype(np.float32) @ Wv
    out = np.empty((B, T, H), np.float32)
    neg = np.float32(-1e30)
    for b in range(B):
        s = (qh[b] @ kh[b].T) / np.float32(np.sqrt(H))
        s = np.where(mask == 0, neg, s)
        s = s - s.max(axis=-1, keepdims=True)
        e = np.exp(s)
        w = e / e.sum(axis=-1, keepdims=True)
        out[b] = w @ vh[b]
    return out


def _w_layout(w, np_in):
    """[D, H] -> [128, DC, H]: partition-major layout for dense DMA."""
    return np.ascontiguousarray(
        w.reshape(DC, 128, H).transpose(1, 0, 2), np_in)


def _make_pat(half):
    """[128, 4, 512] tail-mask patterns; only the first 256 cols are used."""
    tk = np.arange(128)[:, None]
    c = np.arange(2 * TILE)[None, :]
    stair0 = (c >= tk).astype(np.float32)
    stair1 = (c >= 128 + tk).astype(np.float32)
    ones = np.ones((128, 2 * TILE), np.float32)
    zeros = np.zeros((128, 2 * TILE), np.float32)
    if half == 0:
        pats = [stair0, stair1, zeros, zeros]
    else:
        pats = [ones, ones, stair0, stair1]
    return np.stack(pats, axis=1)  # [128, 4, 512]


def _make_in_maps(q, k, v, mask, Wq, Wk, Wv, counts, apply_tail, np_in):
    # kv half-columns owned by core half h: [gg*1024 + h*512, +512) per gg
    half_idx = [
        np.concatenate([np.arange(gg * 1024 + h * 512,
                                  gg * 1024 + h * 512 + 512)
                        for gg in range(NG)])
        for h in range(2)
    ]
    in_maps = []
    metas = []
    for core in range(8):
        b, h = divmod(core, 2)
        tiles = TILES_H0 if h == 0 else TILES_H1
        qT_slab = np.concatenate(
            [q[b, i * TILE:(i + 1) * TILE, :].T for i in tiles], axis=1)
        pat = _make_pat(h) if apply_tail else np.ones(
            (128, 4, 2 * TILE), np.float32)
        kTb = k[b].T
        vTb = v[b].T
        im = {
            "qT": np.ascontiguousarray(qT_slab, np_in),
            "kT": np.ascontiguousarray(kTb[:, half_idx[h]], np_in),
            "vT": np.ascontiguousarray(vTb[:, half_idx[h]], np_in),
            "wq": _w_layout(Wq, np_in), "wk": _w_layout(Wk, np_in),
            "wv": _w_layout(Wv, np_in),
            "pat": np.ascontiguousarray(pat, np_in),
        }
        in_maps.append(im)
        metas.append((b, tiles))
    return in_maps, metas


def kernel(q, k, v, mask, Wq, Wk, Wv):
    from concourse.bass_utils import run_bass_kernel_spmd
    import ml_dtypes

    q = np.ascontiguousarray(q, np.float32)
    k = np.ascontiguousarray(k, np.float32)
    v = np.ascontiguousarray(v, np.float32)
    Wq = np.ascontiguousarray(Wq, np.float32)
    Wk = np.ascontiguousarray(Wk, np.float32)
    Wv = np.ascontiguousarray(Wv, np.float32)
    mask = np.asarray(mask)

    is_tril = bool((mask == np.tril(np.ones((T, T), mask.dtype))).all())
    is_ones = bool((mask == 1).all())
    if not (is_tril or is_ones):
        return _numpy_fallback(q, k, v, mask, Wq, Wk, Wv)

    np_in = ml_dtypes.bfloat16
    counts = COUNTS if is_tril else [NKV] * NPOS
    nc = _get_program(("v6", is_tril), counts, is_tril)

    in_maps, metas = _make_in_maps(
        q, k, v, mask, Wq, Wk, Wv, counts, is_tril, np_in)
    res = run_bass_kernel_spmd(nc, in_maps, list(range(8)))

    out = np.empty((B, T, H), np.float32)
    for c in range(8):
        b, tiles = metas[c]
        oc = res.results[c]["outT"]  # [H+1, TQ]: rows 0..H-1 num, row H den
        slab = (oc[:H, :] / oc[H:H + 1, :]).T  # [TQ, H]
        for p, i in enumerate(tiles):
            out[b, i * TILE:(i + 1) * TILE, :] = \
                slab[p * TILE:(p + 1) * TILE, :]
    return out
